# revision 27
# baseline (speedup 1.0000x reference)
"""Single-head attention (nn_MultiHeadAttention) Trainium2 Bass kernel.

Full inputs: x [4, 2048, 1024], Wq/Wk/Wv/Wo [1024, 1024], biases [1024].
reference:  q = x @ Wq.T + bq ; k,v likewise
            scores = (q @ k.T) / sqrt(1024) ; attn = softmax(scores, -1)
            out = (attn @ v) @ Wo.T + bo

Sharding: 8 cores = 4 batches x 2 query-halves; each core owns 1024
queries and all 2048 keys of its batch.  Keys are stored in per-core
ROLLED order (own 1024 keys first, then the pair-peer's) so the query
rows are literally columns 0:1024 of the key matrix: the x data is
loaded ONCE and shared by the Q/V projections, the scores and the ctx
phases.  Key order only permutes the softmax sum, so results are exact.

Algebraic fusions (host-side weight transforms):
  scores:  q k^T = x (Wq^T Wk) x^T + (bq Wk) x^T + per-query consts that
           cancel in softmax.  A = Wq^T Wk is precomputed on the host, so
           the K projection disappears; the per-key offset o_k = x_k.(bq Wk)
           rides in through the exp's per-partition bias.
  output:  (attn @ (x Wv^T + bv)) Wo^T + bo = attn @ (x (Wo Wv)^T + bc)
           with bc = Wo bv + bo, because the softmax rows sum to 1.  With
           Wvo = Wo Wv precomputed on the host, the ctx matmul yields the
           FINAL output directly — no separate out-projection phase.

V dedup: each core projects VO' = x (Wo Wv)^T + bc only for its OWN 1024
keys, keeps it in SBUF (it directly serves rolled key blocks 0..7 of the
ctx phase), and spills a copy for a pairwise AllGather through a DRAM
bounce.  Only the PEER half is reloaded, via per-rank predicated DMAs
(cond registers loaded from a tiny per-core flags input) — the program
stays SPMD-uniform while each rank pulls the other rank's slot.

Per-core pipeline (all matmul operands bf16, fp32 PSUM accumulation):
  VO phase:  VO'[s,f]  = xq^T Wvo^T + bc   in 4 (f-half, sgroup) chunks
             spill -> AllGather[pair] -> reload peer half   (async)
  QA phase:  QAT[d',q] = A^T xq^T          (d-outer)
  scores:    u[k,q]    = exp((QAT^T x)^T * scale + o_k * scale)
             Z[q]      = sum_k u           (DVE accumulation + gpsimd
                                            cross-partition all-reduce)
  out:       out[f,q]  = (VO'^T u) * (1/Z) (bf16, f-major, host untiles)

DMA plan: each engine's DMA path has ~4 fast in-flight ring slots, ring
completion semaphores lag the data by ~3-5us, and concurrently active
rings share HBM bandwidth, so the start-critical stream of each queue is
at most 4 size-graded fully-contiguous transfers (p-major pre-permuted
on the host where a transfer spans several d-tiles):
  gpsimd: x own-sg0 halves d=0..7 (gates VO chunk 0), flags, bc,
          a_rows 4..7, soff, AllGather, peer-V reload, z round-trip
  sync:   wv f-half-0 graded x4 (gates chunks 0/1), x peer halves d=0..7
          (scores-only, late), VO spills
  scalar: x own-sg1 graded x4 (gates chunk 1), wv f-half-1 (chunk 2+),
          a_rows 0..3, out stores
A 15-matmul warm-up on a zeroed tile fills the otherwise-dead semaphore
-lag window at the start and releases the PE HAM clock throttle before
the first real matmul.
"""

import numpy as np
from contextlib import ExitStack

import ml_dtypes

import concourse.bass as bass
import concourse.bacc as bacc
import concourse.bass_isa as bass_isa
import concourse.mybir as mybir
import concourse.tile as tile
from concourse import bass_utils

F32 = mybir.dt.float32
F32R = mybir.dt.float32r
BF16 = mybir.dt.bfloat16
I32 = mybir.dt.int32
AF = mybir.ActivationFunctionType
ALU = mybir.AluOpType

B, S, D = 4, 2048, 1024
SQ = S // 2  # queries per core
N_CORES = 8
DT_ = D // 128
SCALE = 1.0 / float(np.sqrt(D))

G1DT = BF16   # x, A, qa, wvo  (QA / scores / VO matmuls)
G2DT = BF16   # vo, u          (ctx matmuls)


def build_nc():
    P = 128
    DT = D // P          # contraction tiles (8)
    ET = D // P          # output-dim tiles (8)
    SQW = 512            # query free-dim block
    SQB = SQ // SQW      # (2)
    SKT = S // P         # key tiles (16)
    SOT = SQ // P        # own-key tiles (8)
    NBW = 512            # free-dim block over D for the VO phase
    NB = D // NBW        # (2)

    nc = bacc.Bacc("TRN2", target_bir_lowering=False, debug=False,
                   num_devices=N_CORES)

    # inputs pre-tiled on the host; *p tensors are p-major pre-permuted so
    # each multi-d-tile DMA is one fully-contiguous DRAM run
    xtq1 = nc.dram_tensor("xtq1", [DT, P, SQW], G1DT, kind="ExternalInput")
    xtq2 = nc.dram_tensor("xtq2", [DT, P, SQW], G1DT, kind="ExternalInput")
    xq1a = nc.dram_tensor("xq1a", [P, 2, SQW], G1DT, kind="ExternalInput")
    xq1b = nc.dram_tensor("xq1b", [P, 4, SQW], G1DT, kind="ExternalInput")
    xtp = nc.dram_tensor("xtp", [DT, P, SQ], G1DT, kind="ExternalInput")
    wvh = nc.dram_tensor("wvh", [NB, DT, P, NBW], G1DT, kind="ExternalInput")
    wv1pa = nc.dram_tensor("wv1pa", [P, 4, NBW], G1DT, kind="ExternalInput")
    wv1pb = nc.dram_tensor("wv1pb", [P, 4, NBW], G1DT, kind="ExternalInput")
    a0pa = nc.dram_tensor("a0pa", [P, 2, D], G1DT, kind="ExternalInput")
    a0pb = nc.dram_tensor("a0pb", [P, 2, D], G1DT, kind="ExternalInput")
    a1p = nc.dram_tensor("a1p", [P, 4, D], G1DT, kind="ExternalInput")
    bcd = nc.dram_tensor("bc", [D], F32, kind="ExternalInput")
    soffd = nc.dram_tensor("soff", [S], F32, kind="ExternalInput")
    outd = nc.dram_tensor("out", [ET, SQB, P, SQW], BF16, kind="ExternalOutput")

    def bcast_ap(handle):
        a = handle[:]
        return bass.AP(tensor=a.tensor, offset=a.offset, ap=[[0, P]] + list(a.ap))

    with tile.TileContext(nc) as tc, ExitStack() as top:
        psum = top.enter_context(tc.tile_pool(name="psum", bufs=8, space="PSUM"))
        dram = top.enter_context(tc.tile_pool(name="dram", bufs=1, space="DRAM"))
        singles = top.enter_context(tc.tile_pool(name="singles", bufs=1))
        vb_in = dram.tile([SOT, P, D], G2DT, name="vb_in", tag="vb_in")
        vb_out = dram.tile([SOT, P, D], G2DT, name="vb_out", tag="vb_out")

        # ---- right-side pools, reserved in release order (LIFO top last)
        vp_pool = tc.alloc_tile_pool(name="vp", bufs=SOT, side="right")
        vpeer_tiles = [vp_pool.tile([P, D], G2DT, name=f"vp{i}", tag="vp")
                       for i in range(SOT)]
        u_pool = tc.alloc_tile_pool(name="u", bufs=SKT * SQB, side="right")
        u_tiles = [[None] * SKT for _ in range(SQB)]
        vown_pool = tc.alloc_tile_pool(name="vown", bufs=SOT, side="right")
        vown_tiles = [vown_pool.tile([P, D], G2DT, name=f"vo{i}", tag="vo")
                      for i in range(SOT)]
        zacc_pool = tc.alloc_tile_pool(name="zacc", bufs=SQB, side="right")
        wv_pool = tc.alloc_tile_pool(name="wv", bufs=1, side="right")

        # ---- left-side: xt under qa under a_row (released in reverse)
        xt_pool = tc.alloc_tile_pool(name="xt", bufs=1)
        qa_pool = tc.alloc_tile_pool(name="qa", bufs=ET)
        qa_tiles = [qa_pool.tile([P, SQ], G1DT, name=f"qa{i}", tag="qa")
                    for i in range(ET)]
        a_pool = tc.alloc_tile_pool(name="arow", bufs=1)

        # PE warm-up: fills the dead DMA-semaphore-lag window at the start
        # and releases the HAM clock throttle before real matmuls begin.
        warm_src = singles.tile([P, SQW], G1DT, name="warm", tag="warm")
        nc.vector.memset(warm_src[:], 0)
        warm_ps = psum.tile([P, SQW], F32, name="warm_ps", tag="mm")
        for _ in range(15):
            nc.tensor.matmul(warm_ps, lhsT=warm_src[:, 0:P], rhs=warm_src,
                             start=True, stop=True)

        # ---------------- loads ----------------
        wv_full = wv_pool.tile([P, DT, D], G1DT, name="wv", tag="wv")
        xt_all = xt_pool.tile([P, DT, S], G1DT, name="xt", tag="xt")
        for d in range(DT):
            nc.gpsimd.dma_start(out=xt_all[:, d, 0:SQW], in_=xtq1[d])
            nc.sync.dma_start(out=wv_full[:, d, 0:NBW], in_=wvh[0, d])
        nc.scalar.dma_start(out=xt_all[:, 0, SQW:SQ], in_=xtq2[0])
        nc.scalar.dma_start(out=xt_all[:, 1, SQW:SQ], in_=xtq2[1])
        nc.scalar.dma_start(out=xt_all[:, 2:4, SQW:SQ], in_=xq1a[:])
        nc.scalar.dma_start(out=xt_all[:, 4:DT, SQW:SQ], in_=xq1b[:])
        nc.scalar.dma_start(out=wv_full[:, 0:4, NBW:D], in_=wv1pa[:])
        nc.scalar.dma_start(out=wv_full[:, 4:DT, NBW:D], in_=wv1pb[:])
        bc_bc = singles.tile([P, D], F32, name="bc_bc", tag="bc_bc")
        nc.gpsimd.dma_start(out=bc_bc, in_=bcast_ap(bcd))
        arow_all = a_pool.tile([P, DT, D], G1DT, name="arow", tag="ar")
        nc.sync.dma_start(out=arow_all[:, 0:2, :], in_=a0pa[:])
        nc.sync.dma_start(out=arow_all[:, 2:4, :], in_=a0pb[:])
        nc.gpsimd.dma_start(out=arow_all[:, 4:DT, :], in_=a1p[:])
        # peer key halves: only the scores phase needs them, ~45us in
        for d in range(DT):
            nc.sync.dma_start(out=xt_all[:, d, SQ:S], in_=xtp[d])

        # constants (emitted after the start-critical loads)
        soff_pt = singles.tile([P, SKT], F32, name="soff_pt", tag="soff_pt")
        nc.gpsimd.dma_start(out=soff_pt, in_=soffd[:].rearrange("(t p) -> p t", p=P))
        rz_bc = singles.tile([P, SQ], F32, name="rz_bc", tag="rz_bc")

        # ---------------- VO phase first (own keys only) -------------------
        # Four chunks (fb-half x sgroup); chunk (fb, sg) consumes one DMA
        # queue's stream so the warm matmul stream never outruns the loads.
        for fb in range(NB):
            for sg in range(2):
                pv = [psum.tile([P, NBW], F32, name="mm", tag="mm")
                      for _ in range(4)]
                for d in range(DT):
                    for si in range(4):
                        nc.tensor.matmul(
                            pv[si],
                            lhsT=xt_all[:, d, (sg * 4 + si) * P:(sg * 4 + si + 1) * P],
                            rhs=wv_full[:, d, fb * NBW:(fb + 1) * NBW],
                            start=(d == 0), stop=(d == DT - 1),
                        )
                for si in range(4):
                    s = sg * 4 + si
                    nc.vector.scalar_tensor_tensor(
                        out=vown_tiles[s][:, fb * NBW:(fb + 1) * NBW],
                        in0=pv[si], scalar=1.0,
                        in1=bc_bc[:, fb * NBW:(fb + 1) * NBW],
                        op0=ALU.mult, op1=ALU.add,
                    )
                if fb == NB - 1:
                    for si in range(4):
                        s = sg * 4 + si
                        nc.sync.dma_start(out=vb_in[s], in_=vown_tiles[s])
        # pairwise exchange: AllReduce(add) the own halves, reload the sum,
        # then recover the peer half in place as vsum - vown.  This keeps
        # the program SPMD-uniform with no rank-dependent addressing, and
        # moves only 2MB each way.
        nc.gpsimd.collective_compute(
            "AllReduce",
            ALU.add,
            replica_groups=[[0, 1], [2, 3], [4, 5], [6, 7]],
            ins=[vb_in[:]],
            outs=[vb_out[:]],
        )
        for j in range(SOT):
            nc.gpsimd.dma_start(out=vpeer_tiles[j], in_=vb_out[j])
            nc.vector.tensor_tensor(out=vpeer_tiles[j], in0=vpeer_tiles[j],
                                    in1=vown_tiles[j], op=ALU.subtract)

        # ---------------- QA phase (d-outer) ----------------
        for sb in range(SQB):
            pq = [psum.tile([P, SQW], F32, name="mm", tag="mm") for _ in range(ET)]
            for d in range(DT):
                for et in range(ET):
                    nc.tensor.matmul(
                        pq[et],
                        lhsT=arow_all[:, d, et * P:(et + 1) * P],
                        rhs=xt_all[:, d, sb * SQW:(sb + 1) * SQW],
                        start=(d == 0), stop=(d == DT - 1),
                    )
            for et in range(ET):
                nc.scalar.activation(
                    out=qa_tiles[et][:, sb * SQW:(sb + 1) * SQW],
                    in_=pq[et], func=AF.Copy,
                )
        a_pool.release()

        # ---------------- scores + Z ----------------
        for sk in range(SKT):
            for q in range(SQB):
                ps = psum.tile([P, SQW], F32, name="mm", tag="mm")
                for e in range(ET):
                    nc.tensor.matmul(
                        ps,
                        lhsT=xt_all[:, e, sk * P:(sk + 1) * P],
                        rhs=qa_tiles[e][:, q * SQW:(q + 1) * SQW],
                        start=(e == 0), stop=(e == ET - 1),
                    )
                ut = u_pool.tile([P, SQW], G2DT, name=f"u{q}_{sk}", tag="u")
                nc.scalar.activation(
                    out=ut, in_=ps, func=AF.Exp,
                    bias=soff_pt[:, sk:sk + 1], scale=SCALE,
                )
                u_tiles[q][sk] = ut
                if sk == 0:
                    za = zacc_pool.tile([P, SQW], F32R, name=f"za{q}", tag="za")
                    nc.vector.tensor_copy(za, ut)
                    if q == 0:
                        zacc = [za]
                    else:
                        zacc.append(za)
                else:
                    nc.vector.tensor_tensor(
                        out=zacc[q], in0=zacc[q], in1=ut, op=ALU.add)

        # Z -> 1/Z replicated across partitions, entirely off the PE queue
        for zq in range(SQB):
            zsum = singles.tile([P, SQW], F32, name=f"zsum{zq}", tag=f"zsum{zq}")
            nc.gpsimd.partition_all_reduce(
                zsum[:], zacc[zq][:], P, bass_isa.ReduceOp.add)
            nc.vector.reciprocal(
                out=rz_bc[:, zq * SQW:(zq + 1) * SQW], in_=zsum)

        wv_pool.release()
        zacc_pool.release()
        qa_pool.release()
        xt_pool.release()

        # ---------------- fused ctx/out phase ----------------
        with tc.tile_pool(name="ofly", bufs=4) as o_pool:
            for q in range(SQB):
                for e in range(ET):
                    pc = psum.tile([P, SQW], F32, name="mm", tag="mm")
                    for sk in range(SKT):
                        vt = (vown_tiles[sk] if sk < SOT
                              else vpeer_tiles[sk - SOT])
                        nc.tensor.matmul(
                            pc,
                            lhsT=vt[:, e * P:(e + 1) * P],
                            rhs=u_tiles[q][sk],
                            start=(sk == 0), stop=(sk == SKT - 1),
                        )
                    osb = o_pool.tile([P, SQW], BF16, name="osb", tag="ofly")
                    last = (q == SQB - 1 and e == ET - 1)
                    # split the final tile so the very last store is small
                    strips = ((0, 256), (256, 512)) if last else ((0, SQW),)
                    for lo, hi in strips:
                        nc.vector.tensor_tensor(
                            out=osb[:, lo:hi], in0=pc[:, lo:hi],
                            in1=rz_bc[:, q * SQW + lo:q * SQW + hi],
                            op=ALU.mult)
                        nc.scalar.dma_start(out=outd[e, q, :, lo:hi],
                                            in_=osb[:, lo:hi])
        vown_pool.release()
        u_pool.release()
        vp_pool.release()

    nc.compile()
    return nc


_NC_CACHE = {}


def _get_nc():
    if "nc" not in _NC_CACHE:
        _NC_CACHE["nc"] = build_nc()
    return _NC_CACHE["nc"]


def _cast(a, dt):
    a = np.ascontiguousarray(np.asarray(a, np.float32))
    if dt == BF16:
        return a.astype(ml_dtypes.bfloat16)
    return a


def _tile_rows(m, dt):
    """[D, N] -> contiguous [D//128, 128, N] row-tiles, cast to dt."""
    m = np.asarray(m, np.float32)
    return np.ascontiguousarray(_cast(m, dt).reshape(m.shape[0] // 128, 128, -1))


def make_in_maps(x, Wq, bq, Wk, bk, Wv, bv, Wo, bo):
    x = np.asarray(x, np.float32)
    Wq = np.asarray(Wq, np.float32)
    Wk = np.asarray(Wk, np.float32)
    Wv = np.asarray(Wv, np.float32)
    Wo = np.asarray(Wo, np.float32)
    # A = Wq^T Wk so scores = x A x^T (+ per-key offset from bq, see header)
    ta = _tile_rows(Wq.T @ Wk, G1DT)                 # [DT, 128, D]
    a0pa = np.ascontiguousarray(ta[0:2].transpose(1, 0, 2))
    a0pb = np.ascontiguousarray(ta[2:4].transpose(1, 0, 2))
    a1p = np.ascontiguousarray(ta[4:8].transpose(1, 0, 2))
    # Wvo = Wo Wv folds the output projection into the value path; the
    # matching bias constant is bc = Wo bv + bo (softmax rows sum to 1)
    tw = _tile_rows((Wo @ Wv).T, G1DT)               # [DT, 128, D]
    wvh = np.ascontiguousarray(tw.reshape(DT_, 128, 2, D // 2)
                               .transpose(2, 0, 1, 3))
    wv1pa = np.ascontiguousarray(tw[0:4, :, 512:].transpose(1, 0, 2))
    wv1pb = np.ascontiguousarray(tw[4:8, :, 512:].transpose(1, 0, 2))
    bc = np.ascontiguousarray(Wo @ np.asarray(bv, np.float32)
                              + np.asarray(bo, np.float32))
    ck = np.asarray(bq, np.float32) @ Wk  # [d]

    in_maps = []
    for c in range(N_CORES):
        b, h = c // 2, c % 2
        xb = x[b]  # [S, D]
        rolled = np.concatenate([xb[h * SQ:(h + 1) * SQ],
                                 xb[(1 - h) * SQ:(2 - h) * SQ]], axis=0)
        t = _tile_rows(rolled.T, G1DT)               # [DT, 128, S] rolled keys
        xtq1 = np.ascontiguousarray(t[:, :, 0:512])
        xtq2 = np.ascontiguousarray(t[:, :, 512:1024])
        xq1a = np.ascontiguousarray(t[2:4, :, 512:1024].transpose(1, 0, 2))
        xq1b = np.ascontiguousarray(t[4:8, :, 512:1024].transpose(1, 0, 2))
        xtp = np.ascontiguousarray(t[:, :, 1024:2048])
        soff = np.ascontiguousarray((rolled @ ck) * np.float32(SCALE))
        in_maps.append({
            "xtq1": xtq1, "xtq2": xtq2, "xq1a": xq1a, "xq1b": xq1b,
            "xtp": xtp,
            "wvh": wvh, "wv1pa": wv1pa, "wv1pb": wv1pb,
            "a0pa": a0pa, "a0pb": a0pb, "a1p": a1p,
            "bc": bc, "soff": soff,
        })
    return in_maps


def assemble(results):
    out = np.empty((B, S, D), np.float32)
    for c in range(N_CORES):
        b, h = c // 2, c % 2
        # [8(e), 2(qb), 128(f), 512(q)] tiled, f-major -> [1024 q, 1024 f]
        blk = np.asarray(results[c]["out"], dtype=np.float32)
        out[b, h * SQ:(h + 1) * SQ] = (
            blk.transpose(1, 3, 0, 2).reshape(SQ, D))
    return out


def kernel(x, Wq, bq, Wk, bk, Wv, bv, Wo, bo, **kwargs):
    nc = _get_nc()
    in_maps = make_in_maps(x, Wq, bq, Wk, bk, Wv, bv, Wo, bo)
    res = bass_utils.run_bass_kernel_spmd(nc, in_maps, core_ids=list(range(N_CORES)))
    return assemble(res.results)


# revision 28
# speedup vs baseline: 1.0300x; 1.0300x over previous
"""Single-head attention (nn_MultiHeadAttention) Trainium2 Bass kernel.

Full inputs: x [4, 2048, 1024], Wq/Wk/Wv/Wo [1024, 1024], biases [1024].
reference:  q = x @ Wq.T + bq ; k,v likewise
            scores = (q @ k.T) / sqrt(1024) ; attn = softmax(scores, -1)
            out = (attn @ v) @ Wo.T + bo

Sharding: 8 cores = 4 batches x 2 query-halves; each core owns 1024
queries and all 2048 keys of its batch.  Keys are stored in per-core
ROLLED order (own 1024 keys first, then the pair-peer's) so the query
rows are literally columns 0:1024 of the key matrix: the x data is
loaded ONCE and shared by the Q/V projections, the scores and the ctx
phases.  Key order only permutes the softmax sum, so results are exact.

Algebraic fusions (host-side weight transforms):
  scores:  q k^T = x (Wq^T Wk) x^T + (bq Wk) x^T + per-query consts that
           cancel in softmax.  A = Wq^T Wk is precomputed on the host, so
           the K projection disappears; the per-key offset o_k = x_k.(bq Wk)
           rides in through the exp's per-partition bias.
  output:  (attn @ (x Wv^T + bv)) Wo^T + bo = attn @ (x (Wo Wv)^T + bc)
           with bc = Wo bv + bo, because the softmax rows sum to 1.  With
           Wvo = Wo Wv precomputed on the host, the ctx matmul yields the
           FINAL output directly — no separate out-projection phase.

V dedup: each core projects VO' = x (Wo Wv)^T + bc only for its OWN 1024
keys, keeps it in SBUF (it directly serves rolled key blocks 0..7 of the
ctx phase), and spills a copy for a pairwise AllGather through a DRAM
bounce.  Only the PEER half is reloaded, via per-rank predicated DMAs
(cond registers loaded from a tiny per-core flags input) — the program
stays SPMD-uniform while each rank pulls the other rank's slot.

Per-core pipeline (all matmul operands bf16, fp32 PSUM accumulation):
  VO phase:  VO'[s,f]  = xq^T Wvo^T + bc   in 4 (f-half, sgroup) chunks
             spill -> AllGather[pair] -> reload peer half   (async)
  QA phase:  QAT[d',q] = A^T xq^T          (d-outer)
  scores:    u[k,q]    = exp((QAT^T x)^T * scale + o_k * scale)
             Z[q]      = sum_k u           (DVE accumulation + gpsimd
                                            cross-partition all-reduce)
  out:       out[f,q]  = (VO'^T u) * (1/Z) (bf16, f-major, host untiles)

DMA plan: each engine's DMA path has ~4 fast in-flight ring slots, ring
completion semaphores lag the data by ~3-5us, and concurrently active
rings share HBM bandwidth, so the start-critical stream of each queue is
at most 4 size-graded fully-contiguous transfers (p-major pre-permuted
on the host where a transfer spans several d-tiles):
  gpsimd: x own-sg0 halves d=0..7 (gates VO chunk 0), flags, bc,
          a_rows 4..7, soff, AllGather, peer-V reload, z round-trip
  sync:   wv f-half-0 graded x4 (gates chunks 0/1), x peer halves d=0..7
          (scores-only, late), VO spills
  scalar: x own-sg1 graded x4 (gates chunk 1), wv f-half-1 (chunk 2+),
          a_rows 0..3, out stores
A 15-matmul warm-up on a zeroed tile fills the otherwise-dead semaphore
-lag window at the start and releases the PE HAM clock throttle before
the first real matmul.
"""

import numpy as np
from contextlib import ExitStack

import ml_dtypes

import concourse.bass as bass
import concourse.bacc as bacc
import concourse.bass_isa as bass_isa
import concourse.mybir as mybir
import concourse.tile as tile
from concourse import bass_utils

F32 = mybir.dt.float32
F32R = mybir.dt.float32r
BF16 = mybir.dt.bfloat16
I32 = mybir.dt.int32
AF = mybir.ActivationFunctionType
ALU = mybir.AluOpType

B, S, D = 4, 2048, 1024
SQ = S // 2  # queries per core
N_CORES = 8
DT_ = D // 128
SCALE = 1.0 / float(np.sqrt(D))

G1DT = BF16   # x, A, qa, wvo  (QA / scores / VO matmuls)
G2DT = BF16   # vo, u          (ctx matmuls)


def build_nc():
    P = 128
    DT = D // P          # contraction tiles (8)
    ET = D // P          # output-dim tiles (8)
    SQW = 512            # query free-dim block
    SQB = SQ // SQW      # (2)
    SKT = S // P         # key tiles (16)
    SOT = SQ // P        # own-key tiles (8)
    NBW = 512            # free-dim block over D for the VO phase
    NB = D // NBW        # (2)

    nc = bacc.Bacc("TRN2", target_bir_lowering=False, debug=False,
                   num_devices=N_CORES)

    # inputs pre-tiled on the host; *p tensors are p-major pre-permuted so
    # each multi-d-tile DMA is one fully-contiguous DRAM run
    xtq1 = nc.dram_tensor("xtq1", [DT, P, SQW], G1DT, kind="ExternalInput")
    xtq2 = nc.dram_tensor("xtq2", [DT, P, SQW], G1DT, kind="ExternalInput")
    xq1a = nc.dram_tensor("xq1a", [P, 2, SQW], G1DT, kind="ExternalInput")
    xq1b = nc.dram_tensor("xq1b", [P, 4, SQW], G1DT, kind="ExternalInput")
    xtp = nc.dram_tensor("xtp", [DT, P, SQ], G1DT, kind="ExternalInput")
    wvh = nc.dram_tensor("wvh", [NB, DT, P, NBW], G1DT, kind="ExternalInput")
    wv1pa = nc.dram_tensor("wv1pa", [P, 4, NBW], G1DT, kind="ExternalInput")
    wv1pb = nc.dram_tensor("wv1pb", [P, 4, NBW], G1DT, kind="ExternalInput")
    a0pa = nc.dram_tensor("a0pa", [P, 2, D], G1DT, kind="ExternalInput")
    a0pb = nc.dram_tensor("a0pb", [P, 2, D], G1DT, kind="ExternalInput")
    a1p = nc.dram_tensor("a1p", [P, 4, D], G1DT, kind="ExternalInput")
    bcd = nc.dram_tensor("bc", [D], F32, kind="ExternalInput")
    soffd = nc.dram_tensor("soff", [S], F32, kind="ExternalInput")
    outd = nc.dram_tensor("out", [ET, SQB, P, SQW], BF16, kind="ExternalOutput")

    def bcast_ap(handle):
        a = handle[:]
        return bass.AP(tensor=a.tensor, offset=a.offset, ap=[[0, P]] + list(a.ap))

    with tile.TileContext(nc) as tc, ExitStack() as top:
        psum = top.enter_context(tc.tile_pool(name="psum", bufs=8, space="PSUM"))
        dram = top.enter_context(tc.tile_pool(name="dram", bufs=1, space="DRAM"))
        singles = top.enter_context(tc.tile_pool(name="singles", bufs=1))
        vb_in = dram.tile([SOT, P, D], G2DT, name="vb_in", tag="vb_in")
        vb_out = dram.tile([SOT, P, D], G2DT, name="vb_out", tag="vb_out")

        # ---- right-side pools, reserved in release order (LIFO top last)
        vp_pool = tc.alloc_tile_pool(name="vp", bufs=SOT, side="right")
        vpeer_tiles = [vp_pool.tile([P, D], G2DT, name=f"vp{i}", tag="vp")
                       for i in range(SOT)]
        u_pool = tc.alloc_tile_pool(name="u", bufs=SKT * SQB, side="right")
        u_tiles = [[None] * SKT for _ in range(SQB)]
        vown_pool = tc.alloc_tile_pool(name="vown", bufs=SOT, side="right")
        vown_tiles = [vown_pool.tile([P, D], G2DT, name=f"vo{i}", tag="vo")
                      for i in range(SOT)]
        zacc_pool = tc.alloc_tile_pool(name="zacc", bufs=SQB, side="right")
        wv_pool = tc.alloc_tile_pool(name="wv", bufs=1, side="right")

        # ---- left-side: xt under qa under a_row (released in reverse)
        xt_pool = tc.alloc_tile_pool(name="xt", bufs=1)
        qa_pool = tc.alloc_tile_pool(name="qa", bufs=ET)
        qa_tiles = [qa_pool.tile([P, SQ], G1DT, name=f"qa{i}", tag="qa")
                    for i in range(ET)]
        a_pool = tc.alloc_tile_pool(name="arow", bufs=1)

        # PE warm-up: fills the dead DMA-semaphore-lag window at the start
        # and releases the HAM clock throttle before real matmuls begin.
        warm_src = singles.tile([P, SQW], G1DT, name="warm", tag="warm")
        nc.vector.memset(warm_src[:], 0)
        warm_ps = psum.tile([P, SQW], F32, name="warm_ps", tag="mm")
        for _ in range(15):
            nc.tensor.matmul(warm_ps, lhsT=warm_src[:, 0:P], rhs=warm_src,
                             start=True, stop=True)

        # ---------------- loads ----------------
        wv_full = wv_pool.tile([P, DT, D], G1DT, name="wv", tag="wv")
        xt_all = xt_pool.tile([P, DT, S], G1DT, name="xt", tag="xt")
        for d in range(DT):
            nc.gpsimd.dma_start(out=xt_all[:, d, 0:SQW], in_=xtq1[d])
            nc.sync.dma_start(out=wv_full[:, d, 0:NBW], in_=wvh[0, d])
        nc.scalar.dma_start(out=xt_all[:, 0, SQW:SQ], in_=xtq2[0])
        nc.scalar.dma_start(out=xt_all[:, 1, SQW:SQ], in_=xtq2[1])
        nc.scalar.dma_start(out=xt_all[:, 2:4, SQW:SQ], in_=xq1a[:])
        nc.scalar.dma_start(out=xt_all[:, 4:DT, SQW:SQ], in_=xq1b[:])
        nc.scalar.dma_start(out=wv_full[:, 0:4, NBW:D], in_=wv1pa[:])
        nc.scalar.dma_start(out=wv_full[:, 4:DT, NBW:D], in_=wv1pb[:])
        bc_bc = singles.tile([P, D], F32, name="bc_bc", tag="bc_bc")
        nc.gpsimd.dma_start(out=bc_bc, in_=bcast_ap(bcd))
        arow_all = a_pool.tile([P, DT, D], G1DT, name="arow", tag="ar")
        nc.sync.dma_start(out=arow_all[:, 0:2, :], in_=a0pa[:])
        nc.sync.dma_start(out=arow_all[:, 2:4, :], in_=a0pb[:])
        nc.gpsimd.dma_start(out=arow_all[:, 4:DT, :], in_=a1p[:])
        # peer key halves: only the scores phase needs them, ~45us in
        for d in range(DT):
            nc.sync.dma_start(out=xt_all[:, d, SQ:S], in_=xtp[d])

        # constants (emitted after the start-critical loads)
        soff_pt = singles.tile([P, SKT], F32, name="soff_pt", tag="soff_pt")
        nc.gpsimd.dma_start(out=soff_pt, in_=soffd[:].rearrange("(t p) -> p t", p=P))
        rz_bc = singles.tile([P, SQ], F32, name="rz_bc", tag="rz_bc")

        # ---------------- VO phase first (own keys only) -------------------
        # Two f-half passes, d-outer over BOTH sgroups (8 PSUM banks live):
        # each d-step runs 8 matmuls per (xtq1[d], xtq2[d], wv[d]) triple, so
        # every DMA queue sees half the delivery-rate demand and the d=0
        # tiles are each the FIRST transfer of their queue.
        for fb in range(NB):
            pv = [psum.tile([P, NBW], F32, name="mm", tag="mm")
                  for _ in range(8)]
            for d in range(DT):
                for sg in range(2):
                    for si in range(4):
                        nc.tensor.matmul(
                            pv[sg * 4 + si],
                            lhsT=xt_all[:, d, (sg * 4 + si) * P:(sg * 4 + si + 1) * P],
                            rhs=wv_full[:, d, fb * NBW:(fb + 1) * NBW],
                            start=(d == 0), stop=(d == DT - 1),
                        )
            for s in range(8):
                nc.vector.scalar_tensor_tensor(
                    out=vown_tiles[s][:, fb * NBW:(fb + 1) * NBW],
                    in0=pv[s], scalar=1.0,
                    in1=bc_bc[:, fb * NBW:(fb + 1) * NBW],
                    op0=ALU.mult, op1=ALU.add,
                )
            if fb == NB - 1:
                for s in range(8):
                    nc.sync.dma_start(out=vb_in[s], in_=vown_tiles[s])
        # pairwise exchange: AllReduce(add) the own halves, reload the sum,
        # then recover the peer half in place as vsum - vown.  This keeps
        # the program SPMD-uniform with no rank-dependent addressing, and
        # moves only 2MB each way.
        nc.gpsimd.collective_compute(
            "AllReduce",
            ALU.add,
            replica_groups=[[0, 1], [2, 3], [4, 5], [6, 7]],
            ins=[vb_in[:]],
            outs=[vb_out[:]],
        )
        for j in range(SOT):
            nc.gpsimd.dma_start(out=vpeer_tiles[j], in_=vb_out[j])
            nc.vector.tensor_tensor(out=vpeer_tiles[j], in0=vpeer_tiles[j],
                                    in1=vown_tiles[j], op=ALU.subtract)

        # ---------------- QA phase (d-outer) ----------------
        for sb in range(SQB):
            pq = [psum.tile([P, SQW], F32, name="mm", tag="mm") for _ in range(ET)]
            for d in range(DT):
                for et in range(ET):
                    nc.tensor.matmul(
                        pq[et],
                        lhsT=arow_all[:, d, et * P:(et + 1) * P],
                        rhs=xt_all[:, d, sb * SQW:(sb + 1) * SQW],
                        start=(d == 0), stop=(d == DT - 1),
                    )
            for et in range(ET):
                nc.scalar.activation(
                    out=qa_tiles[et][:, sb * SQW:(sb + 1) * SQW],
                    in_=pq[et], func=AF.Copy,
                )
        a_pool.release()

        # ---------------- scores + Z ----------------
        for sk in range(SKT):
            for q in range(SQB):
                ps = psum.tile([P, SQW], F32, name="mm", tag="mm")
                for e in range(ET):
                    nc.tensor.matmul(
                        ps,
                        lhsT=xt_all[:, e, sk * P:(sk + 1) * P],
                        rhs=qa_tiles[e][:, q * SQW:(q + 1) * SQW],
                        start=(e == 0), stop=(e == ET - 1),
                    )
                ut = u_pool.tile([P, SQW], G2DT, name=f"u{q}_{sk}", tag="u")
                nc.scalar.activation(
                    out=ut, in_=ps, func=AF.Exp,
                    bias=soff_pt[:, sk:sk + 1], scale=SCALE,
                )
                u_tiles[q][sk] = ut
                if sk == 0:
                    za = zacc_pool.tile([P, SQW], F32R, name=f"za{q}", tag="za")
                    nc.vector.tensor_copy(za, ut)
                    if q == 0:
                        zacc = [za]
                    else:
                        zacc.append(za)
                else:
                    nc.vector.tensor_tensor(
                        out=zacc[q], in0=zacc[q], in1=ut, op=ALU.add)

        # Z -> 1/Z replicated across partitions, entirely off the PE queue
        for zq in range(SQB):
            zsum = singles.tile([P, SQW], F32, name=f"zsum{zq}", tag=f"zsum{zq}")
            nc.gpsimd.partition_all_reduce(
                zsum[:], zacc[zq][:], P, bass_isa.ReduceOp.add)
            nc.vector.reciprocal(
                out=rz_bc[:, zq * SQW:(zq + 1) * SQW], in_=zsum)

        wv_pool.release()
        zacc_pool.release()
        qa_pool.release()
        xt_pool.release()

        # ---------------- fused ctx/out phase ----------------
        with tc.tile_pool(name="ofly", bufs=4) as o_pool:
            for q in range(SQB):
                for e in range(ET):
                    pc = psum.tile([P, SQW], F32, name="mm", tag="mm")
                    for sk in range(SKT):
                        vt = (vown_tiles[sk] if sk < SOT
                              else vpeer_tiles[sk - SOT])
                        nc.tensor.matmul(
                            pc,
                            lhsT=vt[:, e * P:(e + 1) * P],
                            rhs=u_tiles[q][sk],
                            start=(sk == 0), stop=(sk == SKT - 1),
                        )
                    osb = o_pool.tile([P, SQW], BF16, name="osb", tag="ofly")
                    last = (q == SQB - 1 and e == ET - 1)
                    # split the final tile so the very last store is small
                    strips = ((0, 256), (256, 512)) if last else ((0, SQW),)
                    for lo, hi in strips:
                        nc.vector.tensor_tensor(
                            out=osb[:, lo:hi], in0=pc[:, lo:hi],
                            in1=rz_bc[:, q * SQW + lo:q * SQW + hi],
                            op=ALU.mult)
                        nc.scalar.dma_start(out=outd[e, q, :, lo:hi],
                                            in_=osb[:, lo:hi])
        vown_pool.release()
        u_pool.release()
        vp_pool.release()

    nc.compile()
    return nc


_NC_CACHE = {}


def _get_nc():
    if "nc" not in _NC_CACHE:
        _NC_CACHE["nc"] = build_nc()
    return _NC_CACHE["nc"]


def _cast(a, dt):
    a = np.ascontiguousarray(np.asarray(a, np.float32))
    if dt == BF16:
        return a.astype(ml_dtypes.bfloat16)
    return a


def _tile_rows(m, dt):
    """[D, N] -> contiguous [D//128, 128, N] row-tiles, cast to dt."""
    m = np.asarray(m, np.float32)
    return np.ascontiguousarray(_cast(m, dt).reshape(m.shape[0] // 128, 128, -1))


def make_in_maps(x, Wq, bq, Wk, bk, Wv, bv, Wo, bo):
    x = np.asarray(x, np.float32)
    Wq = np.asarray(Wq, np.float32)
    Wk = np.asarray(Wk, np.float32)
    Wv = np.asarray(Wv, np.float32)
    Wo = np.asarray(Wo, np.float32)
    # A = Wq^T Wk so scores = x A x^T (+ per-key offset from bq, see header)
    ta = _tile_rows(Wq.T @ Wk, G1DT)                 # [DT, 128, D]
    a0pa = np.ascontiguousarray(ta[0:2].transpose(1, 0, 2))
    a0pb = np.ascontiguousarray(ta[2:4].transpose(1, 0, 2))
    a1p = np.ascontiguousarray(ta[4:8].transpose(1, 0, 2))
    # Wvo = Wo Wv folds the output projection into the value path; the
    # matching bias constant is bc = Wo bv + bo (softmax rows sum to 1)
    tw = _tile_rows((Wo @ Wv).T, G1DT)               # [DT, 128, D]
    wvh = np.ascontiguousarray(tw.reshape(DT_, 128, 2, D // 2)
                               .transpose(2, 0, 1, 3))
    wv1pa = np.ascontiguousarray(tw[0:4, :, 512:].transpose(1, 0, 2))
    wv1pb = np.ascontiguousarray(tw[4:8, :, 512:].transpose(1, 0, 2))
    bc = np.ascontiguousarray(Wo @ np.asarray(bv, np.float32)
                              + np.asarray(bo, np.float32))
    ck = np.asarray(bq, np.float32) @ Wk  # [d]

    in_maps = []
    for c in range(N_CORES):
        b, h = c // 2, c % 2
        xb = x[b]  # [S, D]
        rolled = np.concatenate([xb[h * SQ:(h + 1) * SQ],
                                 xb[(1 - h) * SQ:(2 - h) * SQ]], axis=0)
        t = _tile_rows(rolled.T, G1DT)               # [DT, 128, S] rolled keys
        xtq1 = np.ascontiguousarray(t[:, :, 0:512])
        xtq2 = np.ascontiguousarray(t[:, :, 512:1024])
        xq1a = np.ascontiguousarray(t[2:4, :, 512:1024].transpose(1, 0, 2))
        xq1b = np.ascontiguousarray(t[4:8, :, 512:1024].transpose(1, 0, 2))
        xtp = np.ascontiguousarray(t[:, :, 1024:2048])
        soff = np.ascontiguousarray((rolled @ ck) * np.float32(SCALE))
        in_maps.append({
            "xtq1": xtq1, "xtq2": xtq2, "xq1a": xq1a, "xq1b": xq1b,
            "xtp": xtp,
            "wvh": wvh, "wv1pa": wv1pa, "wv1pb": wv1pb,
            "a0pa": a0pa, "a0pb": a0pb, "a1p": a1p,
            "bc": bc, "soff": soff,
        })
    return in_maps


def assemble(results):
    out = np.empty((B, S, D), np.float32)
    for c in range(N_CORES):
        b, h = c // 2, c % 2
        # [8(e), 2(qb), 128(f), 512(q)] tiled, f-major -> [1024 q, 1024 f]
        blk = np.asarray(results[c]["out"], dtype=np.float32)
        out[b, h * SQ:(h + 1) * SQ] = (
            blk.transpose(1, 3, 0, 2).reshape(SQ, D))
    return out


def kernel(x, Wq, bq, Wk, bk, Wv, bv, Wo, bo, **kwargs):
    nc = _get_nc()
    in_maps = make_in_maps(x, Wq, bq, Wk, bk, Wv, bv, Wo, bo)
    res = bass_utils.run_bass_kernel_spmd(nc, in_maps, core_ids=list(range(N_CORES)))
    return assemble(res.results)
